# revision 15
# baseline (speedup 1.0000x reference)
"""GCN layer h = relu(D^-1/2 A D^-1/2 (x @ W) + b) on 8 Trainium2 cores.

Strategy (node/data parallel, per the sharding hint): dst-nodes are
partitioned across the 8 cores. The host routes each edge to the core
owning its dst ("all-to-all" done at input-sharding time), pre-scaling
the per-edge source payload xs = x[src]*norm_src[src]*norm_dst[dst]
(W is applied AFTER aggregation, by linearity, so only D_IN=64 values
travel per edge). Each core runs one uniform Bass/Tile program:

  per block (<= WW dst nodes, TPB tiles of 128 edges):
    onehot[e, tl, d] = (dst_rel[e, tl] == d)   one DVE is_equal / block
    psum[64f, WWd]  += payload_t.T @ onehot_tl (PE, TPB matmuls)
    agg = psum -> SBUF (DVE), psum2 = W.T @ agg (PE)
    out_blk = relu(psum2 + bias)               (ACT)

Output is feature-major [64, NB*WW] per core; the host unshards
(permutes) back to the full [N, 64]. Falls back to a pure-numpy
implementation on any device-path failure.
"""
import os
import sys
import types

import numpy as np

N = 100000
E = 1200000
D = 64
P = 128
NCORES = 8
NP_CORE = N // NCORES      # nodes per core
WW = 32                    # dst-window (nodes per block)
TPB = 3                    # tiles (of 128 edges) per block
OUT_BATCH = 16
IS_B = 4                   # blocks per batched is_equal
KC_BLOCKS = 32

LAST_EXEC_NS = None
LAST_TRACE = None

_NC_CACHE = {}


def _install_axon_hooks():
    try:
        if 'antenv.axon_hooks' not in sys.modules:
            from trn_agent_boot.trn_boot import _ntff_profile_via_ctypes
            hook = _ntff_profile_via_ctypes('/opt/axon/libaxon_pjrt.so')
            mod = types.ModuleType('antenv.axon_hooks')
            mod.get_axon_ntff_profile_hook = lambda: hook
            mod.set_axon_ntff_profile_hook = lambda h: None
            sys.modules['antenv.axon_hooks'] = mod
        import concourse.bass_utils as bu
        bu.upload_artifacts = lambda tmpdir: f"local:{tmpdir}"
    except Exception:
        pass


def _build_nc(nb):
    from contextlib import ExitStack
    import concourse.tile as tile
    from concourse import bacc, mybir

    nc = bacc.Bacc(None, target_bir_lowering=False)
    tiles = nb * TPB
    kc = KC_BLOCKS * TPB
    assert tiles % kc == 0 and nb % OUT_BATCH == 0

    pay_h = nc.dram_tensor("payload", [P, tiles, D], mybir.dt.bfloat16,
                           kind="ExternalInput")
    rel_h = nc.dram_tensor("dstrel", [P, tiles], mybir.dt.bfloat16,
                           kind="ExternalInput")
    iota_h = nc.dram_tensor("iota", [P, IS_B * TPB, WW], mybir.dt.bfloat16,
                            kind="ExternalInput")
    w_h = nc.dram_tensor("w", [D, D], mybir.dt.bfloat16, kind="ExternalInput")
    b_h = nc.dram_tensor("bias", [D, 1], mybir.dt.float32, kind="ExternalInput")
    out_h = nc.dram_tensor("out", [D, nb * WW], mybir.dt.float32,
                           kind="ExternalOutput")

    with tile.TileContext(nc) as tc, ExitStack() as ctx:
        constp = ctx.enter_context(tc.tile_pool(name="constp", bufs=1))
        payp = ctx.enter_context(tc.tile_pool(name="payp", bufs=4))
        onep = ctx.enter_context(tc.tile_pool(name="onep", bufs=4))
        aggp = ctx.enter_context(tc.tile_pool(name="aggp", bufs=3))
        outp = ctx.enter_context(tc.tile_pool(name="outp", bufs=3))
        psc = ctx.enter_context(tc.tile_pool(name="psc", bufs=4, space="PSUM"))
        pw = ctx.enter_context(tc.tile_pool(name="pw", bufs=2, space="PSUM"))

        iota_t = constp.tile([P, IS_B * TPB, WW], mybir.dt.bfloat16)
        nc.sync.dma_start(out=iota_t[:], in_=iota_h[:, :, :])
        w_t = constp.tile([D, D], mybir.dt.bfloat16)
        nc.sync.dma_start(out=w_t[:], in_=w_h[:, :])
        b_t = constp.tile([D, 1], mybir.dt.float32)
        nc.sync.dma_start(out=b_t[:], in_=b_h[:, :])
        rel_t = constp.tile([P, tiles], mybir.dt.bfloat16)
        nc.sync.dma_start(out=rel_t[:], in_=rel_h[:, :])

        def wstage(g0, ps):
            aggb = aggp.tile([D, OUT_BATCH * WW], mybir.dt.bfloat16)
            nc.scalar.copy(out=aggb[:], in_=ps[:])
            ps2 = pw.tile([D, OUT_BATCH * WW], mybir.dt.float32)
            nc.tensor.matmul(out=ps2[:], lhsT=w_t[:], rhs=aggb[:],
                             start=True, stop=True)
            outt = outp.tile([D, OUT_BATCH * WW], mybir.dt.float32)
            nc.scalar.activation(
                out=outt[:], in_=ps2[:],
                func=mybir.ActivationFunctionType.Relu,
                bias=b_t[:], scale=1.0)
            nc.sync.dma_start(out=out_h[:, g0 * WW:(g0 + OUT_BATCH) * WW],
                              in_=outt[:])

        pay = None
        pay0 = payp.tile([P, IS_B * TPB, D], mybir.dt.bfloat16, tag="pay0")
        nc.sync.dma_start(out=pay0[:], in_=pay_h[:, 0:IS_B * TPB, :])
        first = IS_B * TPB
        ps = None
        oh = None
        for b in range(nb):
            t0 = b * TPB
            if b % IS_B == 0:
                oh = onep.tile([P, IS_B * TPB, WW], mybir.dt.bfloat16)
                nc.vector.tensor_tensor(
                    out=oh[:],
                    in0=rel_t[:, t0:t0 + IS_B * TPB].to_broadcast(
                        [P, IS_B * TPB, WW]),
                    in1=iota_t[:],
                    op=mybir.AluOpType.is_equal)
            if b % OUT_BATCH == 0:
                ps = psc.tile([D, OUT_BATCH * WW], mybir.dt.float32)
            pslice = ps[:, (b % OUT_BATCH) * WW:(b % OUT_BATCH + 1) * WW]
            for tl in range(TPB):
                t = t0 + tl
                if t < first:
                    lhs = pay0[:, t, :]
                else:
                    if (t - first) % kc == 0:
                        pay = payp.tile([P, kc, D], mybir.dt.bfloat16)
                        te = min(t + kc, tiles)
                        nc.sync.dma_start(out=pay[:, 0:te - t, :],
                                          in_=pay_h[:, t:te, :])
                    lhs = pay[:, (t - first) % kc, :]
                nc.tensor.matmul(out=pslice, lhsT=lhs,
                                 rhs=oh[:, (b % IS_B) * TPB + tl, :],
                                 start=(tl == 0), stop=(tl == TPB - 1))
            if b % OUT_BATCH == OUT_BATCH - 1:
                wstage((b // OUT_BATCH) * OUT_BATCH, ps)
    nc.finalize()
    return nc


def _get_nc(nb):
    nc = _NC_CACHE.get(nb)
    if nc is None:
        nc = _build_nc(nb)
        _NC_CACHE[nb] = nc
    return nc


def _host_prep(x, W, b, src, dst):
    """Shard + route edges; build per-core device inputs."""
    import ml_dtypes
    bf16 = ml_dtypes.bfloat16

    deg_out = np.bincount(src, minlength=N).astype(np.float32)
    deg_in = np.bincount(dst, minlength=N).astype(np.float32)
    norm_src = 1.0 / np.sqrt(np.maximum(deg_out, 1.0))
    norm_dst = 1.0 / np.sqrt(np.maximum(deg_in, 1.0))

    order = np.argsort(dst, kind="stable")
    dst_s = dst[order]
    src_s = src[order]

    cap = TPB * P
    cores = []
    max_nb = 0
    bounds = np.searchsorted(dst_s, np.arange(NCORES + 1) * NP_CORE)
    for c in range(NCORES):
        e0, e1 = bounds[c], bounds[c + 1]
        dc = dst_s[e0:e1]
        sc = src_s[e0:e1]
        uniq, inv, cnt = np.unique(dc, return_inverse=True, return_counts=True)
        if cnt.size and cnt.max() > cap:
            raise ValueError("node degree exceeds block capacity")
        blk_of_node = np.empty(uniq.size, np.int32)
        node_rank = np.empty(uniq.size, np.int32)
        nb_c = 0
        acc_e = 0
        acc_n = 0
        for i in range(uniq.size):
            ci = cnt[i]
            if acc_n + 1 > WW or acc_e + ci > cap:
                nb_c += 1
                acc_e = 0
                acc_n = 0
            blk_of_node[i] = nb_c
            node_rank[i] = acc_n
            acc_n += 1
            acc_e += ci
        nb_c += 1
        max_nb = max(max_nb, nb_c)
        cores.append(dict(e0=e0, e1=e1, dc=dc, sc=sc, uniq=uniq, inv=inv,
                          blk_of_node=blk_of_node, node_rank=node_rank))

    nbu = max(OUT_BATCH, IS_B, KC_BLOCKS)
    nb = -(-max_nb // nbu) * nbu
    tiles = nb * TPB

    iota = np.tile(np.arange(WW, dtype=np.float32)[None, None, :],
                   (P, IS_B * TPB, 1)).astype(bf16)
    w_in = np.ascontiguousarray(W, dtype=np.float32).astype(bf16)
    b_in = np.ascontiguousarray(b, dtype=np.float32).reshape(D, 1)

    in_maps = []
    unshard = []
    for c in range(NCORES):
        cc = cores[c]
        payload = np.zeros((P, tiles, D), bf16)
        rel = np.full((P, tiles), 999.0, bf16)
        ne = cc["e1"] - cc["e0"]
        if ne:
            eblk = cc["blk_of_node"][cc["inv"]]
            erank = cc["node_rank"][cc["inv"]]
            blk_sizes = np.bincount(eblk, minlength=nb)
            blk_starts = np.concatenate([[0], np.cumsum(blk_sizes)[:-1]])
            slot = np.arange(ne) - blk_starts[eblk]
            t_all = eblk * TPB + slot // P
            p_all = slot % P
            vals = (x[cc["sc"]] * norm_src[cc["sc"]][:, None]
                    * norm_dst[cc["dc"]][:, None]).astype(bf16)
            payload[p_all, t_all, :] = vals
            rel[p_all, t_all] = erank.astype(np.float32)
        in_maps.append(dict(payload=payload, dstrel=rel, iota=iota,
                            w=w_in, bias=b_in))
        unshard.append(cc)
    return nb, in_maps, unshard


def _kernel_device(x, W, b, src, dst):
    global LAST_EXEC_NS, LAST_TRACE
    _install_axon_hooks()
    from concourse.bass_utils import run_bass_kernel_spmd

    nb, in_maps, unshard = _host_prep(x, W, b, src, dst)
    nc = _get_nc(nb)
    trace = os.environ.get("GCN_TRACE", "") == "1"
    res = run_bass_kernel_spmd(nc, in_maps, core_ids=list(range(NCORES)),
                               trace=trace)
    LAST_EXEC_NS = res.exec_time_ns
    if res.instructions_and_trace is not None:
        LAST_TRACE = res.instructions_and_trace[1]

    out = np.tile(np.maximum(b.astype(np.float32), 0.0), (N, 1))
    for c in range(NCORES):
        cc = unshard[c]
        r = res.results[c]["out"].T            # [nb*WW, 64] node-major
        rows = cc["blk_of_node"] * WW + cc["node_rank"]
        out[cc["uniq"], :] = r[rows, :]

    # spot-check ~512 random nodes against a direct host computation;
    # guards against transient device races.
    rng = np.random.default_rng(12345)
    chk = rng.choice(N, size=512, replace=False)
    so = np.argsort(dst, kind="stable")
    ds, ss = dst[so], src[so]
    deg_out = np.bincount(src, minlength=N).astype(np.float32)
    deg_in = np.bincount(dst, minlength=N).astype(np.float32)
    ns_ = 1.0 / np.sqrt(np.maximum(deg_out, 1.0))
    nd_ = 1.0 / np.sqrt(np.maximum(deg_in, 1.0))
    exp = np.empty((chk.size, D), np.float32)
    for i, n_ in enumerate(chk):
        lo, hi = np.searchsorted(ds, n_), np.searchsorted(ds, n_ + 1)
        agg = (x[ss[lo:hi]] * ns_[ss[lo:hi]][:, None]).sum(axis=0)             if hi > lo else np.zeros(D, np.float32)
        exp[i] = np.maximum((agg * nd_[n_]) @ W + b, 0.0)
    err = np.linalg.norm(out[chk] - exp) / (np.linalg.norm(exp) + 1e-30)
    if not np.isfinite(err) or err > 1.2e-2:
        raise RuntimeError(f"device spot-check failed: rel={err:.3e}")
    return out


def _kernel_numpy(x, W, b, src, dst):
    deg_out = np.bincount(src, minlength=N).astype(np.float32)
    deg_in = np.bincount(dst, minlength=N).astype(np.float32)
    norm_src = 1.0 / np.sqrt(np.maximum(deg_out, 1.0))
    norm_dst = 1.0 / np.sqrt(np.maximum(deg_in, 1.0))
    h = x @ W
    hs = h * norm_src[:, None]
    msg = hs[src]
    agg = np.zeros((x.shape[0], W.shape[1]), np.float32)
    np.add.at(agg, dst, msg)
    out = agg * norm_dst[:, None] + b
    return np.maximum(out, 0.0).astype(np.float32)


def _axon_reset():
    try:
        import ctypes
        lib = ctypes.CDLL('/opt/axon/libaxon_pjrt.so')
        lib.axon_reset.restype = ctypes.c_int64
        return lib.axon_reset() == 0
    except Exception:
        return False


def kernel(x, W, b, src, dst):
    x = np.asarray(x, dtype=np.float32)
    W = np.asarray(W, dtype=np.float32)
    b = np.asarray(b, dtype=np.float32)
    src = np.asarray(src).astype(np.int64)
    dst = np.asarray(dst).astype(np.int64)
    if x.shape != (N, D) or W.shape != (D, D) or src.shape != (E,):
        return _kernel_numpy(x, W, b, src, dst)
    for attempt in range(2):
        try:
            return _kernel_device(x, W, b, src, dst)
        except Exception:
            import traceback
            traceback.print_exc()
            if attempt == 0 and not _axon_reset():
                break
    return _kernel_numpy(x, W, b, src, dst)


# revision 16
# speedup vs baseline: 1.1223x; 1.1223x over previous
"""GCN layer h = relu(D^-1/2 A D^-1/2 (x @ W) + b) on 8 Trainium2 cores.

Strategy (node/data parallel, per the sharding hint): dst-nodes are
partitioned across the 8 cores. The host routes each edge to the core
owning its dst ("all-to-all" done at input-sharding time), pre-scaling
the per-edge source payload xs = x[src]*norm_src[src]*norm_dst[dst]
(W is applied AFTER aggregation, by linearity, so only D_IN=64 values
travel per edge). Each core runs one uniform Bass/Tile program:

  per block (<= WW dst nodes, TPB tiles of 128 edges):
    onehot[e, tl, d] = (dst_rel[e, tl] == d)   one DVE is_equal / block
    psum[64f, WWd]  += payload_t.T @ onehot_tl (PE, TPB matmuls)
    agg = psum -> SBUF (DVE), psum2 = W.T @ agg (PE)
    out_blk = relu(psum2 + bias)               (ACT)

Output is feature-major [64, NB*WW] per core; the host unshards
(permutes) back to the full [N, 64]. Falls back to a pure-numpy
implementation on any device-path failure.
"""
import os
import sys
import types

import numpy as np

N = 100000
E = 1200000
D = 64
P = 128
NCORES = 8
NP_CORE = N // NCORES      # nodes per core
WW = 32                    # dst-window (nodes per block)
TPB = 3                    # tiles (of 128 edges) per block
OUT_BATCH = 32
IS_B = 4                   # blocks per batched is_equal
KC_BLOCKS = 32

LAST_EXEC_NS = None
LAST_TRACE = None

_NC_CACHE = {}


def _install_axon_hooks():
    try:
        if 'antenv.axon_hooks' not in sys.modules:
            from trn_agent_boot.trn_boot import _ntff_profile_via_ctypes
            hook = _ntff_profile_via_ctypes('/opt/axon/libaxon_pjrt.so')
            mod = types.ModuleType('antenv.axon_hooks')
            mod.get_axon_ntff_profile_hook = lambda: hook
            mod.set_axon_ntff_profile_hook = lambda h: None
            sys.modules['antenv.axon_hooks'] = mod
        import concourse.bass_utils as bu
        bu.upload_artifacts = lambda tmpdir: f"local:{tmpdir}"
    except Exception:
        pass


def _build_nc(nb):
    from contextlib import ExitStack
    import concourse.tile as tile
    from concourse import bacc, mybir

    nc = bacc.Bacc(None, target_bir_lowering=False)
    tiles = nb * TPB
    kc = KC_BLOCKS * TPB
    assert tiles % kc == 0 and nb % OUT_BATCH == 0

    pay_h = nc.dram_tensor("payload", [P, tiles, D], mybir.dt.bfloat16,
                           kind="ExternalInput")
    rel_h = nc.dram_tensor("dstrel", [P, tiles], mybir.dt.bfloat16,
                           kind="ExternalInput")
    iota_h = nc.dram_tensor("iota", [P, IS_B * TPB, WW], mybir.dt.bfloat16,
                            kind="ExternalInput")
    w_h = nc.dram_tensor("w", [D, D], mybir.dt.bfloat16, kind="ExternalInput")
    b_h = nc.dram_tensor("bias", [D, 1], mybir.dt.float32, kind="ExternalInput")
    out_h = nc.dram_tensor("out", [D, nb * WW], mybir.dt.float32,
                           kind="ExternalOutput")

    with tile.TileContext(nc) as tc, ExitStack() as ctx:
        constp = ctx.enter_context(tc.tile_pool(name="constp", bufs=1))
        payp = ctx.enter_context(tc.tile_pool(name="payp", bufs=4))
        onep = ctx.enter_context(tc.tile_pool(name="onep", bufs=4))
        aggp = ctx.enter_context(tc.tile_pool(name="aggp", bufs=3))
        outp = ctx.enter_context(tc.tile_pool(name="outp", bufs=3))
        psc = ctx.enter_context(tc.tile_pool(name="psc", bufs=3, space="PSUM"))
        pw = ctx.enter_context(tc.tile_pool(name="pw", bufs=1, space="PSUM"))

        iota_t = constp.tile([P, IS_B * TPB, WW], mybir.dt.bfloat16)
        nc.sync.dma_start(out=iota_t[:], in_=iota_h[:, :, :])
        w_t = constp.tile([D, D], mybir.dt.bfloat16)
        nc.sync.dma_start(out=w_t[:], in_=w_h[:, :])
        b_t = constp.tile([D, 1], mybir.dt.float32)
        nc.sync.dma_start(out=b_t[:], in_=b_h[:, :])
        rel_t = constp.tile([P, tiles], mybir.dt.bfloat16)
        nc.sync.dma_start(out=rel_t[:], in_=rel_h[:, :])

        def wstage(g0, ps):
            aggb = aggp.tile([D, OUT_BATCH * WW], mybir.dt.bfloat16)
            nc.scalar.copy(out=aggb[:], in_=ps[:])
            ps2 = pw.tile([D, OUT_BATCH * WW], mybir.dt.float32)
            half = OUT_BATCH * WW // 2
            nc.tensor.matmul(out=ps2[:, 0:half], lhsT=w_t[:],
                             rhs=aggb[:, 0:half], start=True, stop=True)
            nc.tensor.matmul(out=ps2[:, half:], lhsT=w_t[:],
                             rhs=aggb[:, half:], start=True, stop=True)
            outt = outp.tile([D, OUT_BATCH * WW], mybir.dt.float32)
            nc.scalar.activation(
                out=outt[:], in_=ps2[:],
                func=mybir.ActivationFunctionType.Relu,
                bias=b_t[:], scale=1.0)
            nc.sync.dma_start(out=out_h[:, g0 * WW:(g0 + OUT_BATCH) * WW],
                              in_=outt[:])

        pay = None
        pay0 = payp.tile([P, IS_B * TPB, D], mybir.dt.bfloat16, tag="pay0")
        nc.sync.dma_start(out=pay0[:], in_=pay_h[:, 0:IS_B * TPB, :])
        first = IS_B * TPB
        ps = None
        oh = None
        for b in range(nb):
            t0 = b * TPB
            if b % IS_B == 0:
                oh = onep.tile([P, IS_B * TPB, WW], mybir.dt.bfloat16)
                nc.vector.tensor_tensor(
                    out=oh[:],
                    in0=rel_t[:, t0:t0 + IS_B * TPB].to_broadcast(
                        [P, IS_B * TPB, WW]),
                    in1=iota_t[:],
                    op=mybir.AluOpType.is_equal)
            if b % OUT_BATCH == 0:
                ps = psc.tile([D, OUT_BATCH * WW], mybir.dt.float32)
            pslice = ps[:, (b % OUT_BATCH) * WW:(b % OUT_BATCH + 1) * WW]
            for tl in range(TPB):
                t = t0 + tl
                if t < first:
                    lhs = pay0[:, t, :]
                else:
                    if (t - first) % kc == 0:
                        pay = payp.tile([P, kc, D], mybir.dt.bfloat16)
                        te = min(t + kc, tiles)
                        nc.sync.dma_start(out=pay[:, 0:te - t, :],
                                          in_=pay_h[:, t:te, :])
                    lhs = pay[:, (t - first) % kc, :]
                nc.tensor.matmul(out=pslice, lhsT=lhs,
                                 rhs=oh[:, (b % IS_B) * TPB + tl, :],
                                 start=(tl == 0), stop=(tl == TPB - 1))
            if b % OUT_BATCH == OUT_BATCH - 1:
                wstage((b // OUT_BATCH) * OUT_BATCH, ps)
    nc.finalize()
    return nc


def _get_nc(nb):
    nc = _NC_CACHE.get(nb)
    if nc is None:
        nc = _build_nc(nb)
        _NC_CACHE[nb] = nc
    return nc


def _host_prep(x, W, b, src, dst):
    """Shard + route edges; build per-core device inputs."""
    import ml_dtypes
    bf16 = ml_dtypes.bfloat16

    deg_out = np.bincount(src, minlength=N).astype(np.float32)
    deg_in = np.bincount(dst, minlength=N).astype(np.float32)
    norm_src = 1.0 / np.sqrt(np.maximum(deg_out, 1.0))
    norm_dst = 1.0 / np.sqrt(np.maximum(deg_in, 1.0))

    order = np.argsort(dst, kind="stable")
    dst_s = dst[order]
    src_s = src[order]

    cap = TPB * P
    cores = []
    max_nb = 0
    bounds = np.searchsorted(dst_s, np.arange(NCORES + 1) * NP_CORE)
    for c in range(NCORES):
        e0, e1 = bounds[c], bounds[c + 1]
        dc = dst_s[e0:e1]
        sc = src_s[e0:e1]
        uniq, inv, cnt = np.unique(dc, return_inverse=True, return_counts=True)
        if cnt.size and cnt.max() > cap:
            raise ValueError("node degree exceeds block capacity")
        blk_of_node = np.empty(uniq.size, np.int32)
        node_rank = np.empty(uniq.size, np.int32)
        nb_c = 0
        acc_e = 0
        acc_n = 0
        for i in range(uniq.size):
            ci = cnt[i]
            if acc_n + 1 > WW or acc_e + ci > cap:
                nb_c += 1
                acc_e = 0
                acc_n = 0
            blk_of_node[i] = nb_c
            node_rank[i] = acc_n
            acc_n += 1
            acc_e += ci
        nb_c += 1
        max_nb = max(max_nb, nb_c)
        cores.append(dict(e0=e0, e1=e1, dc=dc, sc=sc, uniq=uniq, inv=inv,
                          blk_of_node=blk_of_node, node_rank=node_rank))

    nbu = max(OUT_BATCH, IS_B, KC_BLOCKS)
    nb = -(-max_nb // nbu) * nbu
    tiles = nb * TPB

    iota = np.tile(np.arange(WW, dtype=np.float32)[None, None, :],
                   (P, IS_B * TPB, 1)).astype(bf16)
    w_in = np.ascontiguousarray(W, dtype=np.float32).astype(bf16)
    b_in = np.ascontiguousarray(b, dtype=np.float32).reshape(D, 1)

    in_maps = []
    unshard = []
    for c in range(NCORES):
        cc = cores[c]
        payload = np.zeros((P, tiles, D), bf16)
        rel = np.full((P, tiles), 999.0, bf16)
        ne = cc["e1"] - cc["e0"]
        if ne:
            eblk = cc["blk_of_node"][cc["inv"]]
            erank = cc["node_rank"][cc["inv"]]
            blk_sizes = np.bincount(eblk, minlength=nb)
            blk_starts = np.concatenate([[0], np.cumsum(blk_sizes)[:-1]])
            slot = np.arange(ne) - blk_starts[eblk]
            t_all = eblk * TPB + slot // P
            p_all = slot % P
            vals = (x[cc["sc"]] * norm_src[cc["sc"]][:, None]
                    * norm_dst[cc["dc"]][:, None]).astype(bf16)
            payload[p_all, t_all, :] = vals
            rel[p_all, t_all] = erank.astype(np.float32)
        in_maps.append(dict(payload=payload, dstrel=rel, iota=iota,
                            w=w_in, bias=b_in))
        unshard.append(cc)
    return nb, in_maps, unshard


def _kernel_device(x, W, b, src, dst):
    global LAST_EXEC_NS, LAST_TRACE
    _install_axon_hooks()
    from concourse.bass_utils import run_bass_kernel_spmd

    nb, in_maps, unshard = _host_prep(x, W, b, src, dst)
    nc = _get_nc(nb)
    trace = os.environ.get("GCN_TRACE", "") == "1"
    res = run_bass_kernel_spmd(nc, in_maps, core_ids=list(range(NCORES)),
                               trace=trace)
    LAST_EXEC_NS = res.exec_time_ns
    if res.instructions_and_trace is not None:
        LAST_TRACE = res.instructions_and_trace[1]

    out = np.tile(np.maximum(b.astype(np.float32), 0.0), (N, 1))
    for c in range(NCORES):
        cc = unshard[c]
        r = res.results[c]["out"].T            # [nb*WW, 64] node-major
        rows = cc["blk_of_node"] * WW + cc["node_rank"]
        out[cc["uniq"], :] = r[rows, :]

    # spot-check ~512 random nodes against a direct host computation;
    # guards against transient device races.
    rng = np.random.default_rng(12345)
    chk = rng.choice(N, size=512, replace=False)
    so = np.argsort(dst, kind="stable")
    ds, ss = dst[so], src[so]
    deg_out = np.bincount(src, minlength=N).astype(np.float32)
    deg_in = np.bincount(dst, minlength=N).astype(np.float32)
    ns_ = 1.0 / np.sqrt(np.maximum(deg_out, 1.0))
    nd_ = 1.0 / np.sqrt(np.maximum(deg_in, 1.0))
    exp = np.empty((chk.size, D), np.float32)
    for i, n_ in enumerate(chk):
        lo, hi = np.searchsorted(ds, n_), np.searchsorted(ds, n_ + 1)
        agg = (x[ss[lo:hi]] * ns_[ss[lo:hi]][:, None]).sum(axis=0)             if hi > lo else np.zeros(D, np.float32)
        exp[i] = np.maximum((agg * nd_[n_]) @ W + b, 0.0)
    err = np.linalg.norm(out[chk] - exp) / (np.linalg.norm(exp) + 1e-30)
    if not np.isfinite(err) or err > 1.2e-2:
        raise RuntimeError(f"device spot-check failed: rel={err:.3e}")
    return out


def _kernel_numpy(x, W, b, src, dst):
    deg_out = np.bincount(src, minlength=N).astype(np.float32)
    deg_in = np.bincount(dst, minlength=N).astype(np.float32)
    norm_src = 1.0 / np.sqrt(np.maximum(deg_out, 1.0))
    norm_dst = 1.0 / np.sqrt(np.maximum(deg_in, 1.0))
    h = x @ W
    hs = h * norm_src[:, None]
    msg = hs[src]
    agg = np.zeros((x.shape[0], W.shape[1]), np.float32)
    np.add.at(agg, dst, msg)
    out = agg * norm_dst[:, None] + b
    return np.maximum(out, 0.0).astype(np.float32)


def _axon_reset():
    try:
        import ctypes
        lib = ctypes.CDLL('/opt/axon/libaxon_pjrt.so')
        lib.axon_reset.restype = ctypes.c_int64
        return lib.axon_reset() == 0
    except Exception:
        return False


def kernel(x, W, b, src, dst):
    x = np.asarray(x, dtype=np.float32)
    W = np.asarray(W, dtype=np.float32)
    b = np.asarray(b, dtype=np.float32)
    src = np.asarray(src).astype(np.int64)
    dst = np.asarray(dst).astype(np.int64)
    if x.shape != (N, D) or W.shape != (D, D) or src.shape != (E,):
        return _kernel_numpy(x, W, b, src, dst)
    for attempt in range(2):
        try:
            return _kernel_device(x, W, b, src, dst)
        except Exception:
            import traceback
            traceback.print_exc()
            if attempt == 0 and not _axon_reset():
                break
    return _kernel_numpy(x, W, b, src, dst)
